# revision 50
# baseline (speedup 1.0000x reference)
"""Trainium2 Bass kernel for nn_BakedAttentionHead.

Reference computation (per row b of query):
    s      = (q @ K^T) / sqrt(D)                      # (B, N)
    e'     = exp(s - max_n s)
    d      = 1 + sum_n e'
    recip  = 16-step sigmoid long-division approx of 1/d
    out    = (e' * recip) @ V

Kernel restructuring (matches the reference to ~5e-3 of output absmax,
vs the 2e-2 gate):
    e      = ES * exp(s)            (ES=1/4 keeps e below fp8e4m3 max 240)
    emax   = max_n e  ( = ES * exp(max_n s), the "+1" of softmax1 in
                        unnormalized units )
    out    = (e @ V) * (1/(emax + sum_n e) - 2^-17/emax)
The reference's 16-step long-division reciprocal equals 1/d truncated to
16 fractional bits (plus sigmoid-soft-edge noise); an exact reciprocal
biased by half an ulp (the -2^-17 term) matches it to ~1.2e-3 of output
scale, measured on the real score distribution.

Precision scheme: every operand of both big matmuls is decomposed into
fp8e4m3 hi + lo parts (lo = fp8(x - fp8(x)), so hi+lo carries ~11
mantissa bits), and each matmul runs as three fp8 DoubleRow matmuls
(hi@hi + hi@lo + lo@hi; lo@lo is ~1e-6 relative and dropped).  DoubleRow
processes two 128-deep contraction planes per instruction at 0.5
cycles/row, so the three terms cost 0.75x the fp32r/bf16 cycles.  q/k/v
are split on the host (also halving input DMA); e hi/lo are made
on-chip: ACT evacuates each mm1 psum tile twice (exp -> f32 staging,
exp -> fp8 hi) and DVE subtracts for lo, so psum recycling depends on
ACT alone.  The row sum chain runs on GPSIMD and the max chain on DVE,
keeping every engine under the PE's tile period during mm1.

Schedule: both mm1 sweeps run back-to-back on the PE, then both mm2
sweeps.  Each pair's row-scale is ready ~2us after its mm1 ends (no
16-step scan), long before its mm2 evacuations, so every mm2 psum tile
is evacuated with the row scale fused into a single DVE op and DMA'd
straight out.  Input DMA interleaves k-hi/k-lo in 512-column chunks in
first-use order so the PE never waits past the pipeline fill.

Sharding: data-parallel over the 8192 query rows -> 8 cores x 1024 rows,
keys/values replicated.
"""

import math

import numpy as np

B, D, N = 8192, 1024, 2048
NCORES = 8
M = B // NCORES            # 1024 query rows per core
NPAIR = 2                  # m "pairs" per core (one mm1 sweep each)
PW = M // NPAIR            # 512 m per pair = mm1 moving free dim
MT = PW // 128             # 4 output m-tiles of 128 rows per pair
NT = N // 128              # 16 n tiles
DJ = D // 256              # 4 DoubleRow contraction steps for mm1
NJ = N // 256              # 8 DoubleRow contraction steps for mm2
SCALE = 0.03125            # D ** -0.5
ES = 0.25                  # e pre-scale: e = ES*exp(s) stays under fp8 max 240
LNES = math.log(ES)
QBIAS = 2.0 ** -17         # half-ulp of the reference's 16-bit long division

_CACHE = {}


def _build(reps=1):
    import concourse.mybir as mybir
    import concourse.tile as tile
    from concourse import bacc
    from concourse.masks import make_identity

    F32 = mybir.dt.float32
    F8 = mybir.dt.float8e4
    AX = mybir.AxisListType
    OP = mybir.AluOpType
    AF = mybir.ActivationFunctionType
    DR = mybir.MatmulPerfMode.DoubleRow

    nc = bacc.Bacc("TRN2", target_bir_lowering=False, debug=False,
                   num_devices=NCORES)
    qh_d = nc.declare_dram_parameter("qTh", [D, M], F8, isOutput=False)
    ql_d = nc.declare_dram_parameter("qTl", [D, M], F8, isOutput=False)
    kh_d = nc.declare_dram_parameter("kTh", [D, N], F8, isOutput=False)
    kl_d = nc.declare_dram_parameter("kTl", [D, N], F8, isOutput=False)
    vh_d = nc.declare_dram_parameter("vh", [N, D], F8, isOutput=False)
    vl_d = nc.declare_dram_parameter("vl", [N, D], F8, isOutput=False)
    out_d = nc.declare_dram_parameter("out", [M, D], F32, isOutput=True)

    qh_ap = qh_d[:].rearrange("(dt p) m -> p dt m", p=128)
    ql_ap = ql_d[:].rearrange("(dt p) m -> p dt m", p=128)
    kh_ap = kh_d[:].rearrange("(dt p) n -> p dt n", p=128)
    kl_ap = kl_d[:].rearrange("(dt p) n -> p dt n", p=128)
    vh_ap = vh_d[:].rearrange("(nt p) do -> p nt do", p=128)
    vl_ap = vl_d[:].rearrange("(nt p) do -> p nt do", p=128)

    with tile.TileContext(nc) as tc:
        with (
            tc.tile_pool(name="res", bufs=1) as res_pool,
            tc.tile_pool(name="e", bufs=2) as e_pool,
            tc.tile_pool(name="ef", bufs=4) as ef_pool,
            tc.tile_pool(name="acc", bufs=2) as acc_pool,
            tc.tile_pool(name="qt", bufs=2) as qt_pool,
            tc.tile_pool(name="stat", bufs=2) as stat_pool,
            tc.tile_pool(name="o", bufs=8) as out_pool,
            tc.tile_pool(name="ps1", bufs=5, space="PSUM") as ps1_pool,
            tc.tile_pool(name="ps2", bufs=3, space="PSUM") as ps2_pool,
        ):
            ident = res_pool.tile([128, 128], F32)
            make_identity(nc, ident[:])
            lnes = res_pool.tile([128, 1], F32)
            nc.vector.memset(lnes[:], LNES)
            # warm-up operand on DVE (ready ~0.5us, before make_identity's
            # Pool chain finishes) so the PE p-state ramp starts ASAP
            wtile = res_pool.tile([128, 128], F32)
            nc.vector.memset(wtile[:], 0.0)

            for rep in range(reps):
                # SP HWDGE queue is FIFO: emit loads in first-use order.
                # mm1's first psum tile consumes kh[n0] + all qh, then kl[n0]
                # + all ql; later tiles consume kh/kl in n order, so those
                # stream as interleaved 512-column chunks.  v is only needed
                # once mm2 starts (~45us in).
                kh = res_pool.tile([128, DJ * 2, N], F8, name="kh", tag="kh")
                kl = res_pool.tile([128, DJ * 2, N], F8, name="kl", tag="kl")
                vh = res_pool.tile([128, NJ * 2, D], F8, name="vh", tag="vh")
                vl = res_pool.tile([128, NJ * 2, D], F8, name="vl", tag="vl")
                # All loads ride the single SP HWDGE queue (the DMA transfer
                # pipe is serial across queues anyway).  DMA descriptors with
                # contiguous runs under 512B move at half rate, so k chunks
                # are at least 512 columns wide.
                qts = []
                qt0h = qt_pool.tile([128, DJ * 2, PW], F8, name="q0h", tag="qh")
                qt0l = qt_pool.tile([128, DJ * 2, PW], F8, name="q0l", tag="ql")
                qt1h = qt_pool.tile([128, DJ * 2, PW], F8, name="q1h", tag="qh")
                qt1l = qt_pool.tile([128, DJ * 2, PW], F8, name="q1l", tag="ql")
                # DMA chain in consumption order.  Pair-0's q-lo rides AFTER
                # the first two k chunk-pairs: its matmuls (each tile's third
                # term) are deferred two tiles via interleaved open psum
                # groups, so the PE conveyor starts earlier.
                nc.sync.dma_start(out=kh[:, :, 0:128], in_=kh_ap[:, :, 0:128])
                nc.sync.dma_start(out=qt0h[:], in_=qh_ap[:, :, 0:PW])
                nc.sync.dma_start(out=kl[:, :, 0:128], in_=kl_ap[:, :, 0:128])
                nc.sync.dma_start(out=kh[:, :, 128:640],
                                  in_=kh_ap[:, :, 128:640])
                nc.sync.dma_start(out=kl[:, :, 128:640],
                                  in_=kl_ap[:, :, 128:640])
                nc.sync.dma_start(out=qt0l[:], in_=ql_ap[:, :, 0:PW])
                qts.append((qt0h, qt0l))
                for n0, n1 in [(640, 1152), (1152, 2048)]:
                    nc.sync.dma_start(out=kh[:, :, n0:n1],
                                      in_=kh_ap[:, :, n0:n1])
                    nc.sync.dma_start(out=kl[:, :, n0:n1],
                                      in_=kl_ap[:, :, n0:n1])
                nc.sync.dma_start(out=qt1h[:], in_=qh_ap[:, :, PW:M])
                nc.sync.dma_start(out=qt1l[:], in_=ql_ap[:, :, PW:M])
                qts.append((qt1h, qt1l))
                for c in range(0, NJ * 2, 4):
                    nc.sync.dma_start(out=vh[:, c:c + 4, :],
                                      in_=vh_ap[:, c:c + 4, :])
                    nc.sync.dma_start(out=vl[:, c:c + 4, :],
                                      in_=vl_ap[:, c:c + 4, :])

                warm_ps = [None]

                def emit_warm(n):
                    """Dummy identity matmuls: keep the PE busy while the
                    first input DMAs land so the p-state ramp (full clock
                    after 3us of continuous execution) is spent on throwaway
                    work and the real matmuls never pay it."""
                    if warm_ps[0] is None:
                        warm_ps[0] = ps2_pool.tile([128, 128], F32,
                                                   name="warm", tag="ps2")
                    for _ in range(n):
                        nc.tensor.matmul(warm_ps[0][:], lhsT=wtile[:],
                                         rhs=wtile[:], start=True, stop=True)

                def emit_mm1(p, d1=0, d2=0, warm_sched=None, splice=None):
                    """scores^T for pair p ([n, m] orientation, 512 m), three
                    fp8 DoubleRow terms accumulated per psum tile; ACT
                    evacuates to e_f32 + e_hi(fp8), DVE makes e_lo, GPSIMD
                    accumulates the sum chain, DVE the max chain.

                    Each tile's psum group accumulates three 4-matmul terms:
                    (kh,qh) immediately, (kl,qh) d1 tiles later, (kh,ql) d2
                    tiles later (interleaved open psum groups).  The deferral
                    matches the serial DMA chain's delivery order kh->qh->
                    kl->ql so the PE conveyor starts as soon as the first kh
                    chunk and qh land."""
                    qth, qtl = qts[p]
                    warm_sched = warm_sched or {}
                    splice = splice or {}
                    e_hi = e_pool.tile([128, NT, PW], F8, name=f"eh{p}",
                                       tag="eh")
                    e_lo = e_pool.tile([128, NT, PW], F8, name=f"el{p}",
                                       tag="el")
                    macc = acc_pool.tile([128, PW], F32, name=f"macc{p}",
                                         tag="macc")
                    sacc = acc_pool.tile([128, PW], F32, name=f"sacc{p}",
                                         tag="sacc")
                    open_ps = {}

                    def term(ps, kt_t, qt_t, nt, start, stop):
                        for j in range(DJ):
                            nc.tensor.matmul(
                                ps[:],
                                lhsT=kt_t[:, 2 * j:2 * j + 2,
                                          nt * 128:(nt + 1) * 128],
                                rhs=qt_t[:, 2 * j:2 * j + 2, :],
                                start=(start and j == 0),
                                stop=(stop and j == DJ - 1),
                                perf_mode=DR,
                            )

                    def a_pass(nt):
                        ps = ps1_pool.tile([128, PW], F32, name=f"s{p}_{nt}",
                                           tag="ps1")
                        term(ps, kh, qth, nt, True, False)
                        open_ps[nt] = ps

                    def b1_pass(nt):
                        term(open_ps[nt], kl, qth, nt, False, False)

                    def b2_pass(nt):
                        ps = open_ps.pop(nt)
                        term(ps, kh, qtl, nt, False, True)
                        e_f = ef_pool.tile([128, PW], F32, name=f"ef{p}_{nt}",
                                           tag="ef")
                        nc.scalar.activation(e_f[:], ps[:], AF.Exp,
                                             scale=SCALE, bias=lnes[:, 0:1])
                        nc.scalar.activation(e_hi[:, nt, :], ps[:], AF.Exp,
                                             scale=SCALE, bias=lnes[:, 0:1])
                        nc.vector.tensor_tensor(
                            out=e_lo[:, nt, :], in0=e_f[:],
                            in1=e_hi[:, nt, :], op=OP.subtract)
                        if nt == 0:
                            nc.gpsimd.tensor_copy(sacc[:], e_f[:])
                            nc.vector.tensor_copy(macc[:], e_f[:])
                        else:
                            nc.gpsimd.tensor_tensor(
                                out=sacc[:], in0=e_f[:], in1=sacc[:],
                                op=OP.add)
                            nc.vector.tensor_tensor(
                                out=macc[:], in0=e_f[:], in1=macc[:],
                                op=OP.max)
                        if nt in splice:
                            splice[nt]()

                    for nt in range(NT + max(d1, d2)):
                        if ("a", nt) in warm_sched:
                            emit_warm(warm_sched[("a", nt)])
                        if nt < NT:
                            a_pass(nt)
                        if d1 <= nt < NT + d1:
                            b1_pass(nt - d1)
                        if d2 <= nt < NT + d2:
                            if ("b", nt - d2) in warm_sched:
                                emit_warm(warm_sched[("b", nt - d2)])
                            b2_pass(nt - d2)
                    return e_hi, e_lo, macc, sacc

                def emit_stats(p, macc, sacc, scale_t):
                    """Cross-partition max/sum of the [128 n, 512 m] stat
                    accumulators via PE transposes, then the fused softmax1
                    scale: 1/(emax + sum) - 2^-17/emax."""
                    emax = stat_pool.tile([128, MT], F32, name=f"mx{p}",
                                          tag="mx")
                    sm = stat_pool.tile([128, MT], F32, name=f"sm{p}",
                                        tag="sm")
                    for c in range(MT):
                        pt = ps1_pool.tile([128, 128], F32, name=f"tm{p}_{c}",
                                           tag="ps1")
                        nc.tensor.transpose(
                            pt[:], macc[:, c * 128:(c + 1) * 128], ident[:])
                        nc.vector.tensor_reduce(
                            emax[:, c:c + 1], pt[:], axis=AX.X, op=OP.max)
                        pt2 = ps1_pool.tile([128, 128], F32, name=f"ts{p}_{c}",
                                            tag="ps1")
                        nc.tensor.transpose(
                            pt2[:], sacc[:, c * 128:(c + 1) * 128], ident[:])
                        nc.vector.tensor_reduce(
                            sm[:, c:c + 1], pt2[:], axis=AX.X, op=OP.add)
                    den = stat_pool.tile([128, MT], F32, name=f"den{p}",
                                         tag="den")
                    rmx = stat_pool.tile([128, MT], F32, name=f"rmx{p}",
                                         tag="rmx")
                    nc.vector.tensor_tensor(out=den[:], in0=emax[:],
                                            in1=sm[:], op=OP.add)
                    nc.vector.reciprocal(den[:], den[:])
                    nc.vector.reciprocal(rmx[:], emax[:])
                    nc.vector.scalar_tensor_tensor(
                        out=scale_t[:], in0=rmx[:], scalar=-QBIAS,
                        in1=den[:], op0=OP.mult, op1=OP.add)

                def emit_mm2(p, e_hi, e_lo, scale_t):
                    """out = e @ V as three fp8 DoubleRow terms per [128,
                    512] psum group (a psum bank holds 512 f32 per
                    partition), evacuated with the row scale fused on DVE
                    and DMA'd straight out.  The very last group of the last
                    pair is split in half so the final evac+store chain
                    after the PE finishes is as short as possible."""
                    chunks = [(c, do * 512, (do + 1) * 512)
                              for c in range(MT) for do in range(2)]
                    if p == NPAIR - 1:
                        c, d0, d1 = chunks.pop()
                        chunks += [(c, d0, d0 + 256), (c, d0 + 256, d1)]
                    for gi, (c, d0, d1) in enumerate(chunks):
                        ps = ps2_pool.tile([128, d1 - d0], F32,
                                           name=f"o{p}_{gi}", tag="ps2")
                        terms = [(e_hi, vh), (e_hi, vl), (e_lo, vh)]
                        nmm = len(terms) * NJ
                        i = 0
                        for e_t, v_t in terms:
                            for j in range(NJ):
                                nc.tensor.matmul(
                                    ps[:],
                                    lhsT=e_t[:, 2 * j:2 * j + 2,
                                             c * 128:(c + 1) * 128],
                                    rhs=v_t[:, 2 * j:2 * j + 2, d0:d1],
                                    start=(i == 0), stop=(i == nmm - 1),
                                    perf_mode=DR,
                                )
                                i += 1
                        ot = out_pool.tile([128, d1 - d0], F32,
                                           name=f"ot{p}_{gi}", tag="ot")
                        nc.vector.tensor_scalar_mul(
                            ot[:], ps[:], scale_t[:, c:c + 1])
                        m0 = p * PW + c * 128
                        nc.sync.dma_start(
                            out=out_d[m0:m0 + 128, d0:d1], in_=ot[:])

                scales = [stat_pool.tile([128, MT], F32, name=f"sc{p}",
                                         tag="sc") for p in range(NPAIR)]
                r0 = emit_mm1(0, d1=2, d2=3,
                              warm_sched={("a", 0): 5, ("a", 1): 2,
                                          ("b", 0): 1})
                # stats(p0) transposes are spliced into mm1(p1)'s sweep so
                # the PE reaches them after pair 0's DVE/GPSIMD stat chains
                # have drained (no PE wait).
                r1 = emit_mm1(1, splice={2: lambda: emit_stats(
                    0, r0[2], r0[3], scales[0])})
                emit_mm2(0, r0[0], r0[1], scales[0])
                emit_stats(1, r1[2], r1[3], scales[1])
                emit_mm2(1, r1[0], r1[1], scales[1])

    nc.compile()
    return nc


def _get_nc():
    if "nc" not in _CACHE:
        _CACHE["nc"] = _build()
    return _CACHE["nc"]


def _split_fp8(x):
    import ml_dtypes

    f8 = ml_dtypes.float8_e4m3
    hi = x.astype(f8)
    lo = (x - hi.astype(np.float32)).astype(f8)
    return np.ascontiguousarray(hi), np.ascontiguousarray(lo)


def kernel(query, keys, values):
    from concourse.bass_utils import run_bass_kernel_spmd

    query = np.ascontiguousarray(query, dtype=np.float32)
    keys = np.ascontiguousarray(keys, dtype=np.float32)
    values = np.ascontiguousarray(values, dtype=np.float32)

    nc = _get_nc()
    kTh, kTl = _split_fp8(np.ascontiguousarray(keys.T))
    vh, vl = _split_fp8(values)
    in_maps = []
    for i in range(NCORES):
        qT = np.ascontiguousarray(query[i * M:(i + 1) * M].T)
        qTh, qTl = _split_fp8(qT)
        in_maps.append({"qTh": qTh, "qTl": qTl, "kTh": kTh, "kTl": kTl,
                        "vh": vh, "vl": vl})
    res = run_bass_kernel_spmd(nc, in_maps, list(range(NCORES)))
    out = np.concatenate([res.results[i]["out"] for i in range(NCORES)], axis=0)
    return np.ascontiguousarray(out, dtype=np.float32)
